# revision 1
# baseline (speedup 1.0000x reference)
"""Trainium2 Bass kernel for sparse CausalSelfAttention (8 full heads W=1024,
8 reduced-qk heads W=256), SPMD over 8 NeuronCores.

Sharding: core c -> batch c//4, head-group g=c%4 (full heads 2g,2g+1 and
reduced heads 2g,2g+1). fp16 activations/weights (fp32 PSUM accumulate),
fused per-512-block loop: project block -> attention q-blocks -> c_proj,
c_proj partials DMA'd straight from PSUM; host sums the 4 partials.
"""

import numpy as np

import concourse.bacc as bacc
import concourse.mybir as mybir
from concourse import bass_utils
from concourse.tile import TileContext

# problem constants (hardcoded; kernel.py must be self-contained)
B, T, C = 2, 2048, 1024
HDIM = 64          # full head dim (and v dim of reduced heads)
RDIM = 32          # reduced qk dim
WF, WR = 1024, 256  # windows
QF, QR = 256, 256   # query-block sizes
N_CORES = 8
NK = C // 128       # k-tiles over C contraction
PV_LAG = 2          # software-pipeline depth: PV matmuls lag scores

F32 = mybir.dt.float32
F16 = mybir.dt.float16

# mask offsets d = i0 - kt*128 that need masking: 1.0 where 0 <= d+f-p < W
MASKF_D = [-128, 0, 896, 1024]   # full heads (Q=256, W=1024)
MASKR_D = [256, 128, 0, -128]    # reduced heads (Q=256, W=256)


def host_masks():
    """[128, 8, 256] fp16: 4 full-head masks (W=1024) then 4 reduced
    (W=256); m[p, i, f] = 1.0 where 0 <= d_i + f - p < W."""
    m = np.zeros((128, 8, QF), np.float16)
    p = np.arange(128)[:, None]
    f = np.arange(QF)[None, :]
    for i, (d, w) in enumerate([(d, WF) for d in MASKF_D]
                               + [(d, WR) for d in MASKR_D]):
        m[:, i, :] = ((d + f - p >= 0) & (d + f - p < w)).astype(np.float16)
    return m


def _emit_body(nc, pools, aps):
    (wpool, xbpool, qkpool, ppool, opool, rpool,
     ps_m, ps_s, ps_y) = pools
    xT, wqkv, wproj, masks, out = aps

    # ---- single merged qkv weight tile (one DMA): cols 0:128 wq | 128:256
    # wk | 256:384 packed wqkr | 384:640 wv ----
    wqkv_sb = wpool.tile([128, NK, 640], F16, tag="wqkv")
    wq_sb = wqkv_sb[:, :, 0:128]
    wk_sb = wqkv_sb[:, :, 128:256]
    wqkr_sb = wqkv_sb[:, :, 256:384]
    wv_sb = wqkv_sb[:, :, 384:640]
    wproj_sb = wpool.tile([128, 2, C], F16, tag="wproj")

    # ---- masks precomputed host-side, one DMA (gpsimd is slow on HW) ----
    m_sb = wpool.tile([128, 8, QF], F16, tag="masks")
    mf_sb = m_sb[:, 0:4, :]
    mr_sb = m_sb[:, 4:8, :]

    # persistent transposed activations [dim-stack, T]
    qTf = qkpool.tile([128, T], F16, tag="qTf")  # rows: hA q (64) | hB q (64)
    kTf = qkpool.tile([128, T], F16, tag="kTf")
    # packed reduced: kTr rows 0:32 krA, 32:64 krB, duplicated at 64:128
    # (matmul lhsT/rhs must share a base partition)
    # qTr rows 0:32 qrA, 32:64 zero | 64:96 zero, 96:128 qrB
    qTr = qkpool.tile([128, T], F16, tag="qTr")
    kTr = qkpool.tile([128, T], F16, tag="kTr")
    nc.vector.memset(qTr[32:64, :], 0.0)
    nc.vector.memset(qTr[64:96, :], 0.0)
    # v values + ones block: [128, T-tile, head, 128] (cols 64:128 = 1.0)
    v_sb = qkpool.tile([128, T // 128, 4, 128], F16, tag="v")
    nc.vector.memset(v_sb[:, :, :, 64:128], 1.0)
    # attention outputs yT (normalized), stacked per pair
    yTf = qkpool.tile([128, T], F16, tag="yTf")
    yTr = qkpool.tile([128, T], F16, tag="yTr")

    xT3 = xT.rearrange("(k p) t -> p k t", p=128)

    def project(tb, xts):
        for w_sb, dsts in (
            (wq_sb, ((slice(0, 128), slice(0, 128), qTf),)),
            (wk_sb, ((slice(0, 128), slice(0, 128), kTf),)),
            (wqkr_sb, ((slice(0, 64), slice(0, 64), kTr),
                       (slice(0, 64), slice(64, 128), kTr),
                       (slice(64, 96), slice(0, 32), qTr),
                       (slice(96, 128), slice(96, 128), qTr))),
        ):
            psum = ps_m.tile([128, 512], F32, tag="m")
            for k in range(NK):
                nc.tensor.matmul(psum[:], w_sb[:, k, :], xts[k],
                                 start=(k == 0), stop=(k == NK - 1))
            sl = slice(tb * 512, (tb + 1) * 512)
            for src_rows, dst_rows, dst in dsts:
                nc.vector.tensor_copy(dst[dst_rows, sl], psum[src_rows, :])
        for tt in range(4):
            gt = tb * 4 + tt  # global T-tile
            psv = ps_m.tile([128, 256], F32, tag="m")
            for k in range(NK):
                nc.tensor.matmul(psv[:], xts[k][:, tt * 128:(tt + 1) * 128],
                                 wv_sb[:, k, :],
                                 start=(k == 0), stop=(k == NK - 1))
            nc.vector.tensor_copy(
                v_sb[:, gt, :, 0:64],
                psv[:].rearrange("p (h d) -> p h d", h=4))

    def attn_block(Q, W, m_sb, mask_d, heads, yT, is_full, qb,
                   mid_cb=None):
        i0 = qb * Q
        kt_lo = max(0, i0 - W + 1) // 128
        kt_hi = (i0 + Q - 1) // 128
        kts = list(range(kt_lo, kt_hi + 1))
        py = ps_y.tile([128, 2, 512], F32, tag="y")
        pend = []  # software pipeline: PV lags scores by PV_LAG k-tiles
        for idx, kt in enumerate(kts):
            d = i0 - kt * 128
            pss = ps_s.tile([128, 2, 512], F32, tag="s")
            ksl = slice(kt * 128, (kt + 1) * 128)
            qsl = slice(i0, i0 + Q)
            if is_full:
                nc.tensor.matmul(pss[:, 0, 0:Q], kTf[0:64, ksl],
                                 qTf[0:64, qsl], start=True, stop=True)
                nc.tensor.matmul(pss[:, 1, 0:Q], kTf[64:128, ksl],
                                 qTf[64:128, qsl], start=True, stop=True)
            else:
                nc.tensor.matmul(pss[:, 0, 0:Q], kTr[0:64, ksl],
                                 qTr[0:64, qsl], start=True, stop=True)
                nc.tensor.matmul(pss[:, 1, 0:Q], kTr[64:128, ksl],
                                 qTr[64:128, qsl], start=True, stop=True)
            p_sb = ppool.tile([128, 2, Q], F16, tag="p")
            nc.scalar.activation(p_sb[:], pss[:, :, 0:Q],
                                 mybir.ActivationFunctionType.Exp)
            if d in mask_d:
                mm = m_sb[:, mask_d.index(d), :].rearrange(
                    "p (a q) -> p a q", a=1).broadcast_to([128, 2, Q])
                nc.vector.tensor_mul(p_sb[:], p_sb[:], mm)
            pend.append((p_sb, kt, idx))
            if len(pend) > PV_LAG:
                q0 = pend.pop(0)
                _emit_pv(py, q0[0], q0[1], heads,
                         first=(q0[2] == 0), last=(q0[2] == len(kts) - 1))
        if mid_cb is not None:
            mid_cb()
        for q0 in pend:
            _emit_pv(py, q0[0], q0[1], heads,
                     first=(q0[2] == 0), last=(q0[2] == len(kts) - 1))
        # normalize: yT rows = py[0:64] * reciprocal(denominator rows)
        r_sb = rpool.tile([64, 2, Q], F32, tag="r")
        nc.vector.reciprocal(r_sb[:], py[64:128, :, 0:Q])
        for h, rows in ((0, slice(0, 64)), (1, slice(64, 128))):
            nc.vector.tensor_mul(yT[rows, i0:i0 + Q], py[0:64, h, 0:Q],
                                 r_sb[:, h, :])

    def _emit_pv(py, p_sb, kt, heads, first, last):
        Q = p_sb.shape[-1]
        nc.tensor.matmul(py[:, 0, 0:Q], v_sb[:, kt, heads[0], :],
                         p_sb[:, 0, :], start=first, stop=last)
        nc.tensor.matmul(py[:, 1, 0:Q], v_sb[:, kt, heads[1], :],
                         p_sb[:, 1, :], start=first, stop=last)

    def cproj_pair(qb):
        # c_proj for the two 128-row T-tiles covered by q-block qb
        o_sb = opool.tile([128, 2, C], F16, tag="o")
        for j in range(2):
            tt = 2 * qb + j
            tsl = slice(tt * 128, (tt + 1) * 128)
            for nb in range(2):
                nsl = slice(nb * 512, (nb + 1) * 512)
                pso = ps_m.tile([128, 512], F32, tag="m")
                nc.tensor.matmul(pso[:], yTf[:, tsl], wproj_sb[:, 0, nsl],
                                 start=True, stop=False)
                nc.tensor.matmul(pso[:], yTr[:, tsl], wproj_sb[:, 1, nsl],
                                 start=False, stop=True)
                if nb == 0:
                    nc.scalar.copy(o_sb[:, j, nsl], pso[:])
                else:
                    nc.vector.tensor_copy(o_sb[:, j, nsl], pso[:])
        nc.sync.dma_start(
            out[qb * 256:(qb + 1) * 256, :].rearrange("(j p) m -> p j m",
                                                      p=128), o_sb[:])

    # ---- fused per-512-block loop ----
    xtbs = [None] * 4
    wqkv3 = wqkv.rearrange("(k p) m -> p k m", p=128)
    for tb in range(T // 512):
        sl = slice(tb * 512, (tb + 1) * 512)
        if tb == 0:
            # weights first (one merged DMA), x block 0 in two chunks so
            # the first matmuls can start at the halfway point
            nc.sync.dma_start(wqkv_sb[:, 0:2, :], wqkv3[:, 0:2, :])
            xtb = xbpool.tile([128, NK, 512], F16, tag="xtb")
            nc.sync.dma_start(xtb[:, 0:2, :], xT3[:, 0:2, sl])
            nc.sync.dma_start(wqkv_sb[:, 2:NK, :], wqkv3[:, 2:NK, :])
            nc.sync.dma_start(xtb[:, 2:NK, :], xT3[:, 2:NK, sl])
            nc.sync.dma_start(m_sb[:], masks[:])
            nc.sync.dma_start(wproj_sb[:],
                              wproj.rearrange("(k p) m -> p k m", p=128))
            xtbs[0] = xtb
        xtb = xtbs[tb]
        xts = [xtb[:, k, :] for k in range(NK)]
        project(tb, xts)
        if tb + 1 < 4:  # prefetch next x block behind the projections
            nsl = slice((tb + 1) * 512, (tb + 2) * 512)
            xtb = xbpool.tile([128, NK, 512], F16, tag="xtb")
            nc.sync.dma_start(xtb[:], xT3[:, :, nsl])
            xtbs[tb + 1] = xtb
        qbs = (2 * tb, 2 * tb + 1)
        attn_block(QF, WF, mf_sb, MASKF_D, (0, 1), yTf, True, qbs[0])
        if tb > 0:
            cproj_pair(qbs[0] - 2)  # needs full+red of 2 q-blocks ago
        attn_block(QF, WF, mf_sb, MASKF_D, (0, 1), yTf, True, qbs[1])
        for sub in range(2):
            qb = qbs[sub]
            if qb >= 1 and sub == 0:
                cproj_pair(qb - 1)
            mid_cb = None
            if qb == 7:
                # last pair: yTf contribution inside red(7)'s score shadow,
                # yTr contribution + store at the very end
                psos = []

                def mid_cb():
                    for j in range(2):
                        tsl = slice((14 + j) * 128, (15 + j) * 128)
                        for nb in range(2):
                            nsl = slice(nb * 512, (nb + 1) * 512)
                            pso = ps_m.tile([128, 512], F32, tag="m")
                            nc.tensor.matmul(pso[:], yTf[:, tsl],
                                             wproj_sb[:, 0, nsl],
                                             start=True, stop=False)
                            psos.append((pso, tsl, nsl, nb))
            attn_block(QR, WR, mr_sb, MASKR_D, (2, 3), yTr, False, qb,
                       mid_cb=mid_cb)
            if qb == 6:
                cproj_pair(6)
    for j in range(2):
        tt = 14 + j
        o_sb = opool.tile([128, C], F16, tag="o2")
        for nb in range(2):
            pso, tsl, nsl, _ = psos[2 * j + nb]
            nc.tensor.matmul(pso[:], yTr[:, tsl], wproj_sb[:, 1, nsl],
                             start=False, stop=True)
            if nb == 0:
                nc.scalar.copy(o_sb[:, nsl], pso[:])
            else:
                nc.vector.tensor_copy(o_sb[:, nsl], pso[:])
            nc.sync.dma_start(out[tt * 128:(tt + 1) * 128, nsl],
                              o_sb[:, nsl])


def _build_nc(reps=1):
    nc = bacc.Bacc(trn_type="TRN2", target_bir_lowering=False, debug=False,
                   num_devices=1)

    xT = nc.dram_tensor("xT", [C, T], F16, kind="ExternalInput").ap()
    wqkv = nc.dram_tensor("wqkv", [C, 640], F16, kind="ExternalInput").ap()
    wproj = nc.dram_tensor("wproj", [256, C], F16, kind="ExternalInput").ap()
    masks = nc.dram_tensor("masks", [128, 8 * QF], F16,
                           kind="ExternalInput").ap()
    out = nc.dram_tensor("o", [T, C], F16, kind="ExternalOutput").ap()
    aps = (xT, wqkv, wproj, masks, out)

    with TileContext(nc) as tc:
        with (
            tc.tile_pool(name="wpool", bufs=1) as wpool,
            tc.tile_pool(name="xbpool", bufs=2) as xbpool,
            tc.tile_pool(name="qk", bufs=1) as qkpool,
            tc.tile_pool(name="ppool", bufs=PV_LAG + 3) as ppool,
            tc.tile_pool(name="opool", bufs=3) as opool,
            tc.tile_pool(name="rpool", bufs=4) as rpool,
            tc.tile_pool(name="ps_m", bufs=2, space="PSUM") as ps_m,
            tc.tile_pool(name="ps_s", bufs=2, space="PSUM") as ps_s,
            tc.tile_pool(name="ps_y", bufs=1, space="PSUM") as ps_y,
        ):
            pools = (wpool, xbpool, qkpool, ppool, opool, rpool,
                     ps_m, ps_s, ps_y)
            for _ in range(reps):
                _emit_body(nc, pools, aps)

    nc.compile()
    return nc


_NC_CACHE = {}


def _get_nc(reps=1):
    if reps not in _NC_CACHE:
        _NC_CACHE[reps] = _build_nc(reps)
    return _NC_CACHE[reps]


_MASKS = None


def make_in_maps(x, w_qkv_full, w_qk_red, w_v_red, w_proj):
    global _MASKS
    if _MASKS is None:
        _MASKS = np.ascontiguousarray(host_masks().reshape(128, 8 * QF))
    x = np.asarray(x, np.float32)
    w_qkv_full = np.asarray(w_qkv_full, np.float32)
    w_qk_red = np.asarray(w_qk_red, np.float32)
    w_v_red = np.asarray(w_v_red, np.float32)
    w_proj = np.asarray(w_proj, np.float32)
    sf = np.float32(1.0 / np.sqrt(HDIM))
    sr = np.float32(1.0 / np.sqrt(RDIM))
    in_maps = []
    for c in range(N_CORES):
        b, g = divmod(c, 4)
        hA, hB = 2 * g, 2 * g + 1
        wq = np.concatenate([w_qkv_full[:, 64 * hA:64 * hA + 64],
                             w_qkv_full[:, 64 * hB:64 * hB + 64]], 1) * sf
        wk = np.concatenate([w_qkv_full[:, 512 + 64 * hA:512 + 64 * hA + 64],
                             w_qkv_full[:, 512 + 64 * hB:512 + 64 * hB + 64]], 1)
        # packed reduced projection: rows 0:32 krA | 32:64 krB | qrA | qrB
        wqkr = np.concatenate(
            [w_qk_red[:, 256 + 32 * hA:256 + 32 * hA + 32],
             w_qk_red[:, 256 + 32 * hB:256 + 32 * hB + 32],
             w_qk_red[:, 32 * hA:32 * hA + 32] * sr,
             w_qk_red[:, 32 * hB:32 * hB + 32] * sr], 1)
        wv = np.concatenate([w_qkv_full[:, 1024 + 64 * hA:1024 + 64 * hA + 64],
                             w_qkv_full[:, 1024 + 64 * hB:1024 + 64 * hB + 64],
                             w_v_red[:, 64 * hA:64 * hA + 64],
                             w_v_red[:, 64 * hB:64 * hB + 64]], 1)
        wp = np.concatenate([w_proj[64 * hA:64 * hA + 64, :],
                             w_proj[64 * hB:64 * hB + 64, :],
                             w_proj[512 + 64 * hA:512 + 64 * hA + 64, :],
                             w_proj[512 + 64 * hB:512 + 64 * hB + 64, :]], 0)
        wqkv = np.concatenate([wq, wk, wqkr, wv], 1)
        in_maps.append({
            "xT": np.ascontiguousarray(x[b].T).astype(np.float16),
            "wqkv": np.ascontiguousarray(wqkv).astype(np.float16),
            "wproj": np.ascontiguousarray(wp).astype(np.float16),
            "masks": _MASKS,
        })
    return in_maps


def kernel(x, w_qkv_full, w_qk_red, w_v_red, w_proj):
    nc = _get_nc()
    in_maps = make_in_maps(x, w_qkv_full, w_qk_red, w_v_red, w_proj)
    r = bass_utils.run_bass_kernel_spmd(nc, in_maps,
                                        core_ids=list(range(N_CORES)),
                                        trace=False)
    outs = [r.results[c]["o"] for c in range(N_CORES)]
    y = np.zeros((B, T, C), np.float32)
    for b in range(B):
        for j in range(4):
            y[b] += np.asarray(outs[4 * b + j], np.float32)
    return y



# revision 8
# speedup vs baseline: 1.0552x; 1.0552x over previous
"""Trainium2 Bass kernel for sparse CausalSelfAttention (8 full heads W=1024,
8 reduced-qk heads W=256), SPMD over 8 NeuronCores.

Sharding: core c -> batch c//4, head-group g=c%4 (full heads 2g,2g+1 and
reduced heads 2g,2g+1). fp16 activations/weights (fp32 PSUM accumulate).

v2: live-slice attention blocking (Q=512 full / Q=256 reduced) — score, exp
and PV instructions cover only the in-window column range of each key tile;
the two band edges are handled by two shared 128x128 triangle masks applied
with strided two-region DVE ops. Score matmuls for the head pair run
concurrently via PE row tiling (contraction 64/32 at base partitions 0/64 and
0/32). Reduced-head q/k layout is packed at partitions 0:64 (no zero padding
or duplication).
"""

import numpy as np

import concourse.bacc as bacc
import concourse.mybir as mybir
from concourse import bass_utils
from concourse.ap import AP
from concourse.tile import TileContext

# problem constants (hardcoded; kernel.py must be self-contained)
B, T, C = 2, 2048, 1024
HDIM = 64           # full head dim (and v dim of reduced heads)
RDIM = 32           # reduced qk dim
WF, WR = 1024, 256  # windows
QF, QR = 512, 256   # query-block sizes
N_CORES = 8
NK = C // 128       # k-tiles over C contraction
PV_LAG = 2          # software-pipeline depth: PV matmuls lag exp

F32 = mybir.dt.float32
F16 = mybir.dt.float16
EXP = mybir.ActivationFunctionType.Exp


def host_masks():
    """[128, 2, 128] fp16: [:,0,c] upper triangle keep c<p, [:,1,c] lower
    keep c>=p (c = local query col within the 128-wide edge strip)."""
    p = np.arange(128)[:, None]
    c = np.arange(128)[None, :]
    m = np.zeros((128, 2, 128), np.float16)
    m[:, 0, :] = (c < p).astype(np.float16)
    m[:, 1, :] = (c >= p).astype(np.float16)
    return m


def _emit_body(nc, pools, aps, dbg=None):
    (wpool, xbpool, qkpool, pfpool, prpool, opool, rpool,
     ps_m, ps_s, ps_y) = pools
    xT, wqkv, wproj, masks, out = aps

    # ---- single merged qkv weight tile: cols 0:128 wq | 128:256 wk |
    # 256:384 wqkr (krA krB qrA qrB) | 384:640 wv ----
    wqkv_sb = wpool.tile([128, NK, 640], F16, tag="wqkv")
    wq_sb = wqkv_sb[:, :, 0:128]
    wk_sb = wqkv_sb[:, :, 128:256]
    wqkr_sb = wqkv_sb[:, :, 256:384]
    wv_sb = wqkv_sb[:, :, 384:640]
    wproj_sb = wpool.tile([128, 2, C], F16, tag="wproj")
    m_sb = wpool.tile([128, 2, 128], F16, tag="masks")

    # persistent transposed activations [dim-stack, T]
    qTf = qkpool.tile([128, T], F16, tag="qTf")  # rows: hA q (64) | hB q (64)
    kTf = qkpool.tile([128, T], F16, tag="kTf")
    qTr = qkpool.tile([64, T], F16, tag="qTr")   # rows: qrA (32) | qrB (32)
    kTr = qkpool.tile([64, T], F16, tag="kTr")
    # v values + ones block: [128, T-tile, head, 128] (cols 64:128 = 1.0)
    v_sb = qkpool.tile([128, T // 128, 4, 128], F16, tag="v")
    nc.vector.memset(v_sb[:, :, :, 64:128], 1.0)
    # attention outputs yT (normalized), stacked per pair
    yTf = qkpool.tile([128, T], F16, tag="yTf")
    yTr = qkpool.tile([128, T], F16, tag="yTr")

    xT3 = xT.rearrange("(k p) t -> p k t", p=128)
    wqkv3 = wqkv.rearrange("(k p) m -> p k m", p=128)

    def project(tb, xts):
        sl = slice(tb * 512, (tb + 1) * 512)
        for w_sb, dsts in (
            (wq_sb, ((slice(0, 128), qTf[:, sl]),)),
            (wk_sb, ((slice(0, 128), kTf[:, sl]),)),
            (wqkr_sb, ((slice(0, 64), kTr[:, sl]),
                       (slice(64, 128), qTr[:, sl]))),
        ):
            psum = ps_m.tile([128, 512], F32, tag="m")
            for k in range(NK):
                nc.tensor.matmul(psum[:], w_sb[:, k, :], xts[k],
                                 start=(k == 0), stop=(k == NK - 1))
            for src_rows, dst in dsts:
                nc.vector.tensor_copy(dst, psum[src_rows, :])
        for tt in range(4):
            gt = tb * 4 + tt  # global T-tile
            psv = ps_m.tile([128, 256], F32, tag="m")
            for k in range(NK):
                nc.tensor.matmul(psv[:], xts[k][:, tt * 128:(tt + 1) * 128],
                                 wv_sb[:, k, :],
                                 start=(k == 0), stop=(k == NK - 1))
            nc.vector.tensor_copy(
                v_sb[:, gt, :, 0:64],
                psv[:].rearrange("p (h d) -> p h d", h=4))

    def emit_mask_pair(pb, idx_a, col, mi, Q):
        # one strided op over regions (idx_a, col:col+128) and
        # (idx_a+1, col+128:col+256), multiplied by triangle mask mi
        ap = [list(p) for p in pb.ap]
        pstride, idx_stride, h_stride = ap[0][0], ap[1][0], ap[2][0]
        cust = AP(pb.tensor, pb.offset + idx_a * idx_stride + col,
                  [[pstride, 128], [idx_stride + 128, 2], [h_stride, 2],
                   [1, 128]])
        mm = m_sb[:, mi, :].rearrange("p (a b q) -> p a b q", a=1, b=1)
        nc.vector.tensor_mul(cust, cust, mm.broadcast_to([128, 2, 2, 128]))

    def attn(qb, Q, W, hw, heads, kT, qT, yT, ppool, ptag, nmax):
        i0 = Q * qb
        kt_lo = max(0, i0 - W + 1) // 128
        kt_hi = (i0 + Q - 1) // 128
        kts = list(range(kt_lo, kt_hi + 1))
        n = len(kts)
        pb = ppool.tile([128, nmax, 2, Q], F16, tag=ptag)
        py = ps_y.tile([128, 2, 512], F32, tag="y")
        info = []
        mask_at = {}
        for idx, kt in enumerate(kts):
            d = i0 - 128 * kt
            lo = max(0, -d)
            hi = min(Q, W + 128 - d)
            info.append((idx, kt, lo, hi))
            u, l = W - d, -d
            if 0 <= u < Q and u % 256 == 0:
                mask_at.setdefault(idx + 1, []).append((idx, u, 0))
            if 0 <= l < Q and l % 256 == 0:
                mask_at.setdefault(idx + 1, []).append((idx, l, 1))

        # ready_at[idx]: loop position at which p[idx] is final (after its
        # exp, and after the pair mask op covering it, if any)
        ready_at = {idx: idx for idx, _, _, _ in info}
        for key, lst in mask_at.items():
            for idx_a, _, _ in lst:
                ready_at[idx_a] = key
                ready_at[idx_a + 1] = key
        # the first PV matmul in the accumulation group must cover the full
        # extent later PVs touch (per-bank has_written clear): pick the first
        # fully-live kt and emit its PV before any other
        idx_ff = next(idx for idx, kt, lo, hi in info if lo == 0 and hi == Q)
        last_pv = next(i for i, _, _, _ in reversed(info) if i != idx_ff) \
            if n > 1 else idx_ff

        def emit_pv(idx, kt, lo, hi):
            for h in (0, 1):
                nc.tensor.matmul(py[:, h, lo:hi], v_sb[:, kt, heads[h], :],
                                 pb[:, idx, h, lo:hi],
                                 start=(idx == idx_ff), stop=(idx == last_pv))

        pend = []
        pv_started = False
        for idx, kt, lo, hi in info:
            pss = ps_s.tile([128, 2, 512], F32, tag="s")
            ksl = slice(kt * 128, (kt + 1) * 128)
            qsl = slice(i0 + lo, i0 + hi)
            nc.tensor.matmul(pss[:, 0, lo:hi], kT[0:hw, ksl], qT[0:hw, qsl],
                             start=True, stop=True)
            nc.tensor.matmul(pss[:, 1, lo:hi], kT[hw:2 * hw, ksl],
                             qT[hw:2 * hw, qsl], start=True, stop=True)
            nc.scalar.activation(pb[:, idx, :, lo:hi], pss[:, :, lo:hi], EXP)
            for idx_a, col, mi in mask_at.get(idx, ()):
                emit_mask_pair(pb, idx_a, col, mi, Q)
            if idx != idx_ff:
                pend.append((idx, kt, lo, hi))
            if not pv_started and idx >= ready_at[idx_ff]:
                emit_pv(*info[idx_ff])
                pv_started = True
            if pv_started:
                while len(pend) > PV_LAG:
                    emit_pv(*pend.pop(0))
        if not pv_started:
            emit_pv(*info[idx_ff])
        for e in pend:
            emit_pv(*e)
        # normalize: yT rows = py[0:64] * reciprocal(denominator rows)
        r_sb = rpool.tile([64, 2, Q], F32, tag="r" + ptag)
        nc.vector.reciprocal(r_sb[:], py[64:128, :, 0:Q])
        qsl = slice(i0, i0 + Q)
        nc.vector.tensor_mul(yT[0:64, qsl], py[0:64, 0, 0:Q], r_sb[:, 0, :])
        nc.vector.tensor_mul(yT[64:128, qsl], py[0:64, 1, 0:Q], r_sb[:, 1, :])

    def cproj(tb):
        o_sb = opool.tile([128, 4, C], F16, tag="o")
        for j in range(4):
            tt = 4 * tb + j
            tsl = slice(tt * 128, (tt + 1) * 128)
            for nb in range(2):
                nsl = slice(nb * 512, (nb + 1) * 512)
                pso = ps_m.tile([128, 512], F32, tag="m")
                nc.tensor.matmul(pso[:], yTf[:, tsl], wproj_sb[:, 0, nsl],
                                 start=True, stop=False)
                nc.tensor.matmul(pso[:], yTr[:, tsl], wproj_sb[:, 1, nsl],
                                 start=False, stop=True)
                if (2 * j + nb) % 2 == 0:
                    nc.scalar.copy(o_sb[:, j, nsl], pso[:])
                else:
                    nc.vector.tensor_copy(o_sb[:, j, nsl], pso[:])
        nc.sync.dma_start(
            out[tb * 512:(tb + 1) * 512, :].rearrange("(j p) m -> p j m",
                                                      p=128), o_sb[:])

    # ---- fused per-512-block loop ----
    xtbs = [None] * 4
    for tb in range(T // 512):
        sl = slice(tb * 512, (tb + 1) * 512)
        if tb == 0:
            # weights first (one merged DMA), x block 0 in two chunks so
            # the first matmuls can start at the halfway point
            nc.sync.dma_start(wqkv_sb[:, 0:2, :], wqkv3[:, 0:2, :])
            xtb = xbpool.tile([128, NK, 512], F16, tag="xtb")
            nc.sync.dma_start(xtb[:, 0:2, :], xT3[:, 0:2, sl])
            nc.sync.dma_start(wqkv_sb[:, 2:NK, :], wqkv3[:, 2:NK, :])
            nc.sync.dma_start(xtb[:, 2:NK, :], xT3[:, 2:NK, sl])
            nc.sync.dma_start(m_sb[:],
                              masks.rearrange("p (a q) -> p a q", a=2))
            nc.sync.dma_start(wproj_sb[:],
                              wproj.rearrange("(k p) m -> p k m", p=128))
            xtbs[0] = xtb
        xtb = xtbs[tb]
        xts = [xtb[:, k, :] for k in range(NK)]
        project(tb, xts)
        if tb + 1 < 4:  # prefetch next x block behind the projections
            nsl = slice((tb + 1) * 512, (tb + 2) * 512)
            nxtb = xbpool.tile([128, NK, 512], F16, tag="xtb")
            nc.sync.dma_start(nxtb[:], xT3[:, :, nsl])
            xtbs[tb + 1] = nxtb
        attn(tb, QF, WF, 64, (0, 1), kTf, qTf, yTf, pfpool, "pf", 12)
        if tb > 0:
            cproj(tb - 1)
        attn(2 * tb, QR, WR, 32, (2, 3), kTr, qTr, yTr, prpool, "pr", 4)
        attn(2 * tb + 1, QR, WR, 32, (2, 3), kTr, qTr, yTr, prpool, "pr", 4)
    cproj(3)
    if dbg is not None:
        for name, tile in (("dqTf", qTf), ("dkTf", kTf), ("dqTr", qTr),
                           ("dkTr", kTr), ("dyTf", yTf), ("dyTr", yTr)):
            nc.sync.dma_start(dbg[name], tile[:])
        nc.sync.dma_start(dbg["dv"], v_sb[:].rearrange("p a h q -> p (a h q)"))


def _build_nc(reps=1, debug_outs=False):
    nc = bacc.Bacc(trn_type="TRN2", target_bir_lowering=False, debug=False,
                   num_devices=1)

    xT = nc.dram_tensor("xT", [C, T], F16, kind="ExternalInput").ap()
    wqkv = nc.dram_tensor("wqkv", [C, 640], F16, kind="ExternalInput").ap()
    wproj = nc.dram_tensor("wproj", [256, C], F16, kind="ExternalInput").ap()
    masks = nc.dram_tensor("masks", [128, 256], F16,
                           kind="ExternalInput").ap()
    out = nc.dram_tensor("o", [T, C], F16, kind="ExternalOutput").ap()
    aps = (xT, wqkv, wproj, masks, out)
    dbg = None
    if debug_outs:
        dbg = {}
        for name, shape in (("dqTf", [128, T]), ("dkTf", [128, T]),
                            ("dqTr", [64, T]), ("dkTr", [64, T]),
                            ("dyTf", [128, T]), ("dyTr", [128, T]),
                            ("dv", [128, T * 4])):
            dbg[name] = nc.dram_tensor(name, shape, F16,
                                       kind="ExternalOutput").ap()

    with TileContext(nc) as tc:
        with (
            tc.tile_pool(name="wpool", bufs=1) as wpool,
            tc.tile_pool(name="xbpool", bufs=2) as xbpool,
            tc.tile_pool(name="qk", bufs=1) as qkpool,
            tc.tile_pool(name="pf", bufs=2) as pfpool,
            tc.tile_pool(name="pr", bufs=2) as prpool,
            tc.tile_pool(name="opool", bufs=2) as opool,
            tc.tile_pool(name="rpool", bufs=2) as rpool,
            tc.tile_pool(name="ps_m", bufs=2, space="PSUM") as ps_m,
            tc.tile_pool(name="ps_s", bufs=2, space="PSUM") as ps_s,
            tc.tile_pool(name="ps_y", bufs=1, space="PSUM") as ps_y,
        ):
            pools = (wpool, xbpool, qkpool, pfpool, prpool, opool, rpool,
                     ps_m, ps_s, ps_y)
            for _ in range(reps):
                _emit_body(nc, pools, aps, dbg=dbg)

    nc.compile()
    return nc


_NC_CACHE = {}


def _get_nc(reps=1):
    if reps not in _NC_CACHE:
        _NC_CACHE[reps] = _build_nc(reps)
    return _NC_CACHE[reps]


_MASKS = None


def make_in_maps(x, w_qkv_full, w_qk_red, w_v_red, w_proj):
    global _MASKS
    if _MASKS is None:
        _MASKS = np.ascontiguousarray(host_masks().reshape(128, 256))
    x = np.asarray(x, np.float32)
    w_qkv_full = np.asarray(w_qkv_full, np.float32)
    w_qk_red = np.asarray(w_qk_red, np.float32)
    w_v_red = np.asarray(w_v_red, np.float32)
    w_proj = np.asarray(w_proj, np.float32)
    sf = np.float32(1.0 / np.sqrt(HDIM))
    sr = np.float32(1.0 / np.sqrt(RDIM))
    in_maps = []
    for c in range(N_CORES):
        b, g = divmod(c, 4)
        hA, hB = 2 * g, 2 * g + 1
        wq = np.concatenate([w_qkv_full[:, 64 * hA:64 * hA + 64],
                             w_qkv_full[:, 64 * hB:64 * hB + 64]], 1) * sf
        wk = np.concatenate([w_qkv_full[:, 512 + 64 * hA:512 + 64 * hA + 64],
                             w_qkv_full[:, 512 + 64 * hB:512 + 64 * hB + 64]], 1)
        # packed reduced projection: rows 0:32 krA | 32:64 krB | qrA | qrB
        wqkr = np.concatenate(
            [w_qk_red[:, 256 + 32 * hA:256 + 32 * hA + 32],
             w_qk_red[:, 256 + 32 * hB:256 + 32 * hB + 32],
             w_qk_red[:, 32 * hA:32 * hA + 32] * sr,
             w_qk_red[:, 32 * hB:32 * hB + 32] * sr], 1)
        wv = np.concatenate([w_qkv_full[:, 1024 + 64 * hA:1024 + 64 * hA + 64],
                             w_qkv_full[:, 1024 + 64 * hB:1024 + 64 * hB + 64],
                             w_v_red[:, 64 * hA:64 * hA + 64],
                             w_v_red[:, 64 * hB:64 * hB + 64]], 1)
        wp = np.concatenate([w_proj[64 * hA:64 * hA + 64, :],
                             w_proj[64 * hB:64 * hB + 64, :],
                             w_proj[512 + 64 * hA:512 + 64 * hA + 64, :],
                             w_proj[512 + 64 * hB:512 + 64 * hB + 64, :]], 0)
        wqkv = np.concatenate([wq, wk, wqkr, wv], 1)
        in_maps.append({
            "xT": np.ascontiguousarray(x[b].T).astype(np.float16),
            "wqkv": np.ascontiguousarray(wqkv).astype(np.float16),
            "wproj": np.ascontiguousarray(wp).astype(np.float16),
            "masks": _MASKS,
        })
    return in_maps


def kernel(x, w_qkv_full, w_qk_red, w_v_red, w_proj):
    nc = _get_nc()
    in_maps = make_in_maps(x, w_qkv_full, w_qk_red, w_v_red, w_proj)
    r = bass_utils.run_bass_kernel_spmd(nc, in_maps,
                                        core_ids=list(range(N_CORES)),
                                        trace=False)
    outs = [r.results[c]["o"] for c in range(N_CORES)]
    y = np.zeros((B, T, C), np.float32)
    for b in range(B):
        for j in range(4):
            y[b] += np.asarray(outs[4 * b + j], np.float32)
    return y


# revision 13
# speedup vs baseline: 3.5705x; 3.3835x over previous
"""Trainium2 Bass kernel for sparse CausalSelfAttention (8 full heads W=1024,
8 reduced-qk heads W=256), SPMD over 8 NeuronCores.

Sharding: core c -> batch c//4, head-group g=c%4 (full heads 2g,2g+1 and
reduced heads 2g,2g+1). fp16 activations/weights (fp32 PSUM accumulate).

v3: live-slice attention blocking (Q=512 full / Q=256 reduced) — score, exp
and PV instructions cover only the in-window column range of each key tile;
band edges handled by two shared 128x128 triangle masks applied with strided
two-region DVE ops. Score matmuls for the head pair run concurrently via PE
row tiling. The two reduced sub-blocks of each 512-T slab share one PSUM
accumulator and one normalize. Projection/cproj matmul chains are interleaved
into the attention phase boundaries (engine streams execute in emission
order, so PE work must be woven in manually where exp/normalize would stall).
"""

import numpy as np

import concourse.bacc as bacc
import concourse.mybir as mybir
from concourse import bass_utils
from concourse.ap import AP
from concourse.tile import TileContext

# problem constants (hardcoded; kernel.py must be self-contained)
B, T, C = 2, 2048, 1024
HDIM = 64           # full head dim (and v dim of reduced heads)
RDIM = 32           # reduced qk dim
WF, WR = 1024, 256  # windows
QF, QR = 512, 256   # query-block sizes
N_CORES = 8
NK = C // 128       # k-tiles over C contraction
PV_LAG = 2          # software-pipeline depth: PV matmuls lag exp

F32 = mybir.dt.float32
F16 = mybir.dt.float16
EXP = mybir.ActivationFunctionType.Exp
MASKS_ON_POOL = False  # apply band-edge masks on GpSimd instead of DVE


def host_masks():
    """[128, 2, 128] fp16: [:,0,c] upper triangle keep c<p, [:,1,c] lower
    keep c>=p (c = local query col within the 128-wide edge strip)."""
    p = np.arange(128)[:, None]
    c = np.arange(128)[None, :]
    m = np.zeros((128, 2, 128), np.float16)
    m[:, 0, :] = (c < p).astype(np.float16)
    m[:, 1, :] = (c >= p).astype(np.float16)
    return m


def _emit_body(nc, pools, aps, dbg=None):
    (wpool, xbpool, qkpool, pfpool, prpool, opool, rpool,
     ps_m, ps_s, ps_y) = pools
    xT, wqkv, wproj, masks, out = aps

    # ---- single merged qkv weight tile: cols 0:128 wq | 128:256 wk |
    # 256:384 wqkr (krA krB qrA qrB) | 384:640 wv ----
    wqkv_sb = wpool.tile([128, NK, 640], F16, tag="wqkv")
    wq_sb = wqkv_sb[:, :, 0:128]
    wk_sb = wqkv_sb[:, :, 128:256]
    wqkr_sb = wqkv_sb[:, :, 256:384]
    wv_sb = wqkv_sb[:, :, 384:640]
    wproj_sb = wpool.tile([128, 2, C], F16, tag="wproj")
    m_sb = wpool.tile([128, 2, 128], F16, tag="masks")

    # persistent transposed activations [dim-stack, T]
    qTf = qkpool.tile([128, T], F16, tag="qTf")  # rows: hA q (64) | hB q (64)
    kTf = qkpool.tile([128, T], F16, tag="kTf")
    qTr = qkpool.tile([64, T], F16, tag="qTr")   # rows: qrA (32) | qrB (32)
    kTr = qkpool.tile([64, T], F16, tag="kTr")
    # v values + ones block: [128, T-tile, head, 128] (cols 64:128 = 1.0)
    v_sb = qkpool.tile([128, T // 128, 4, 128], F16, tag="v")
    nc.gpsimd.memset(v_sb[:, :, :, 64:128], 1.0)
    # attention outputs yT (normalized), stacked per pair
    yTf = qkpool.tile([128, T], F16, tag="yTf")
    yTr = qkpool.tile([128, T], F16, tag="yTr")

    xT3 = xT.rearrange("(k p) t -> p k t", p=128)
    wqkv3 = wqkv.rearrange("(k p) m -> p k m", p=128)

    def chain_qk(tb, w_sb, dsts):
        # one projection slab: psum = w.T @ x block, evacuated to dsts
        sl = slice(tb * 512, (tb + 1) * 512)
        xtb = xtbs[tb]
        psum = ps_m.tile([128, 512], F32, tag="m")
        for k in range(NK):
            nc.tensor.matmul(psum[:], w_sb[:, k, :], xtb[:, k, :],
                             start=(k == 0), stop=(k == NK - 1))
        for src_rows, dst in dsts:
            nc.vector.tensor_copy(dst[:, sl], psum[src_rows, :])

    def chain_v(tb, tt):
        gt = tb * 4 + tt  # global T-tile
        xtb = xtbs[tb]
        psv = ps_m.tile([128, 256], F32, tag="m")
        for k in range(NK):
            nc.tensor.matmul(psv[:], xtb[:, k, tt * 128:(tt + 1) * 128],
                             wv_sb[:, k, :],
                             start=(k == 0), stop=(k == NK - 1))
        nc.vector.tensor_copy(v_sb[:, gt, :, 0:64],
                              psv[:].rearrange("p (h d) -> p h d", h=4))

    def emit_mask_pair(pb, idx_a, col, mi):
        # one strided op over regions (idx_a, col:col+128) and
        # (idx_a+1, col+128:col+256), multiplied by triangle mask mi
        ap = [list(p) for p in pb.ap]
        pstride, idx_stride, h_stride = ap[0][0], ap[1][0], ap[2][0]
        cust = AP(pb.tensor, pb.offset + idx_a * idx_stride + col,
                  [[pstride, 128], [idx_stride + 128, 2], [h_stride, 2],
                   [1, 128]])
        mm = m_sb[:, mi, :].rearrange("p (a b q) -> p a b q", a=1, b=1)
        eng = nc.gpsimd if MASKS_ON_POOL else nc.vector
        eng.tensor_mul(cust, cust, mm.broadcast_to([128, 2, 2, 128]))

    def attn_sub(i0, Q, W, hw, heads, kT, qT, pb, py, off,
                 first_start, set_stop):
        """Scores+exp+mask+PV for one query sub-block into py[:, :, off:off+Q].
        first_start: this sub owns the PSUM has_written clear (its first PV
        uses start=True). set_stop: emit stop=True on the last PV."""
        kt_lo = max(0, i0 - W + 1) // 128
        kt_hi = (i0 + Q - 1) // 128
        kts = list(range(kt_lo, kt_hi + 1))
        n = len(kts)
        info = []
        mask_at = {}
        for idx, kt in enumerate(kts):
            d = i0 - 128 * kt
            lo = max(0, -d)
            hi = min(Q, W + 128 - d)
            info.append((idx, kt, lo, hi))
            u, l = W - d, -d
            if 0 <= u < Q and u % 256 == 0:
                mask_at.setdefault(idx + 1, []).append((idx, u, 0))
            if 0 <= l < Q and l % 256 == 0:
                mask_at.setdefault(idx + 1, []).append((idx, l, 1))
        # ready_at[idx]: loop position at which p[idx] is final (after its
        # exp, and after the pair mask op covering it, if any)
        ready_at = {idx: idx for idx, _, _, _ in info}
        for key, lst in mask_at.items():
            for idx_a, _, _ in lst:
                ready_at[idx_a] = key
                ready_at[idx_a + 1] = key
        # the first PV matmul must cover the full extent later PVs touch
        # (the has_written clear is per-bank): pick the first fully-live kt
        idx_ff = next(idx for idx, kt, lo, hi in info if lo == 0 and hi == Q)
        last_pv = next(i for i, _, _, _ in reversed(info) if i != idx_ff) \
            if n > 1 else idx_ff

        def emit_pv(idx, kt, lo, hi):
            for h in (0, 1):
                nc.tensor.matmul(
                    py[:, h, off + lo:off + hi], v_sb[:, kt, heads[h], :],
                    pb[:, idx, h, lo:hi],
                    start=(first_start and idx == idx_ff),
                    stop=(set_stop and idx == last_pv),
                    skip_group_check=True)

        pend = []
        pv_started = False
        for idx, kt, lo, hi in info:
            pss = ps_s.tile([128, 2, 512], F32, tag="s")
            ksl = slice(kt * 128, (kt + 1) * 128)
            qsl = slice(i0 + lo, i0 + hi)
            nc.tensor.matmul(pss[:, 0, lo:hi], kT[0:hw, ksl], qT[0:hw, qsl],
                             start=True, stop=True)
            nc.tensor.matmul(pss[:, 1, lo:hi], kT[hw:2 * hw, ksl],
                             qT[hw:2 * hw, qsl], start=True, stop=True)
            nc.scalar.activation(pb[:, idx, :, lo:hi], pss[:, :, lo:hi], EXP)
            for idx_a, col, mi in mask_at.get(idx, ()):
                emit_mask_pair(pb, idx_a, col, mi)
            if idx != idx_ff:
                pend.append((idx, kt, lo, hi))
            if not pv_started and idx >= ready_at[idx_ff]:
                emit_pv(*info[idx_ff])
                pv_started = True
            if pv_started:
                while len(pend) > PV_LAG:
                    emit_pv(*pend.pop(0))
        if not pv_started:
            emit_pv(*info[idx_ff])
        for e in pend:
            emit_pv(*e)

    def normalize(py, yT, c0, width, rtag):
        r_sb = rpool.tile([64, 2, width], F32, tag=rtag)
        nc.vector.reciprocal(r_sb[:], py[64:128, :, 0:width])
        qsl = slice(c0, c0 + width)
        nc.vector.tensor_mul(yT[0:64, qsl], py[0:64, 0, 0:width],
                             r_sb[:, 0, :])
        nc.vector.tensor_mul(yT[64:128, qsl], py[0:64, 1, 0:width],
                             r_sb[:, 1, :])

    def full_attn(qb):
        pb = pfpool.tile([128, 12, 2, QF], F16, tag="pf")
        py = ps_y.tile([128, 2, 512], F32, tag="y")
        attn_sub(QF * qb, QF, WF, 64, (0, 1), kTf, qTf, pb, py, 0,
                 True, True)
        normalize(py, yTf, QF * qb, QF, "rf")

    def red_pair(tb):
        py = ps_y.tile([128, 2, 512], F32, tag="y")
        for sub in (0, 1):
            pb = prpool.tile([128, 4, 2, QR], F16, tag="pr")
            attn_sub(QR * (2 * tb + sub), QR, WR, 32, (2, 3), kTr, qTr,
                     pb, py, 256 * sub, sub == 0, sub == 1)
        normalize(py, yTr, 512 * tb, 512, "rr")

    def cproj_pair(pp):
        # c_proj for T-tiles 2pp, 2pp+1
        o_sb = opool.tile([128, 2, C], F16, tag="o")
        for j in range(2):
            tt = 2 * pp + j
            tsl = slice(tt * 128, (tt + 1) * 128)
            for nb in range(2):
                nsl = slice(nb * 512, (nb + 1) * 512)
                pso = ps_m.tile([128, 512], F32, tag="m")
                nc.tensor.matmul(pso[:], yTf[:, tsl], wproj_sb[:, 0, nsl],
                                 start=True, stop=False)
                nc.tensor.matmul(pso[:], yTr[:, tsl], wproj_sb[:, 1, nsl],
                                 start=False, stop=True)
                if nb == 0:
                    nc.scalar.copy(o_sb[:, j, nsl], pso[:])
                else:
                    nc.vector.tensor_copy(o_sb[:, j, nsl], pso[:])
        nc.sync.dma_start(
            out[pp * 256:(pp + 1) * 256, :].rearrange("(j p) m -> p j m",
                                                      p=128), o_sb[:])

    # ---- fused per-512-block loop, proj/cproj chains woven into the
    # attention phase boundaries ----
    xtbs = [None] * 4
    qk_dsts = {
        "wq": ((slice(0, 128), qTf),),
        "wk": ((slice(0, 128), kTf),),
        "wqkr": ((slice(0, 64), kTr), (slice(64, 128), qTr)),
    }
    for tb in range(T // 512):
        sl = slice(tb * 512, (tb + 1) * 512)
        if tb == 0:
            # stage by chain: wq columns + x block first so the wq chain
            # starts earliest, then wk, then the rest
            xtb = xbpool.tile([128, NK, 512], F16, tag="xtb")
            nc.sync.dma_start(wqkv_sb[:, :, 0:128], wqkv3[:, :, 0:128])
            nc.sync.dma_start(xtb[:, 0:4, :], xT3[:, 0:4, sl])
            nc.sync.dma_start(xtb[:, 4:NK, :], xT3[:, 4:NK, sl])
            nc.sync.dma_start(wqkv_sb[:, :, 128:256], wqkv3[:, :, 128:256])
            nc.sync.dma_start(wqkv_sb[:, :, 256:640], wqkv3[:, :, 256:640])
            nc.sync.dma_start(m_sb[:],
                              masks.rearrange("p (a q) -> p a q", a=2))
            nc.sync.dma_start(wproj_sb[:],
                              wproj.rearrange("(k p) m -> p k m", p=128))
            xtbs[0] = xtb
            chain_qk(0, wq_sb, qk_dsts["wq"])
            chain_qk(0, wk_sb, qk_dsts["wk"])
            chain_qk(0, wqkr_sb, qk_dsts["wqkr"])
            for tt in range(4):
                chain_v(0, tt)
        if tb + 1 < 4:  # prefetch next x block
            nsl = slice((tb + 1) * 512, (tb + 2) * 512)
            nxtb = xbpool.tile([128, NK, 512], F16, tag="xtb")
            nc.sync.dma_start(nxtb[:], xT3[:, :, nsl])
            xtbs[tb + 1] = nxtb
        full_attn(tb)
        if tb + 1 < 4:  # q/k projections of the next block fill the
            chain_qk(tb + 1, wq_sb, qk_dsts["wq"])      # normalize window
            chain_qk(tb + 1, wk_sb, qk_dsts["wk"])
        else:
            cproj_pair(2)  # dep-free cproj fills the last slab's windows
            cproj_pair(3)
        red_pair(tb)
        if tb == 1:
            cproj_pair(0)
            cproj_pair(1)
        elif tb == 3:
            cproj_pair(4)
            cproj_pair(5)
        if tb + 1 < 4:
            chain_qk(tb + 1, wqkr_sb, qk_dsts["wqkr"])
            for tt in range(4):
                chain_v(tb + 1, tt)
    cproj_pair(6)
    cproj_pair(7)
    if dbg is not None:
        for name, tile in (("dqTf", qTf), ("dkTf", kTf), ("dqTr", qTr),
                           ("dkTr", kTr), ("dyTf", yTf), ("dyTr", yTr)):
            nc.sync.dma_start(dbg[name], tile[:])
        nc.sync.dma_start(dbg["dv"], v_sb[:].rearrange("p a h q -> p (a h q)"))


def _build_nc(reps=1, debug_outs=False):
    nc = bacc.Bacc(trn_type="TRN2", target_bir_lowering=False, debug=False,
                   num_devices=1)

    xT = nc.dram_tensor("xT", [C, T], F16, kind="ExternalInput").ap()
    wqkv = nc.dram_tensor("wqkv", [C, 640], F16, kind="ExternalInput").ap()
    wproj = nc.dram_tensor("wproj", [256, C], F16, kind="ExternalInput").ap()
    masks = nc.dram_tensor("masks", [128, 256], F16,
                           kind="ExternalInput").ap()
    out = nc.dram_tensor("o", [T, C], F16, kind="ExternalOutput").ap()
    aps = (xT, wqkv, wproj, masks, out)
    dbg = None
    if debug_outs:
        dbg = {}
        for name, shape in (("dqTf", [128, T]), ("dkTf", [128, T]),
                            ("dqTr", [64, T]), ("dkTr", [64, T]),
                            ("dyTf", [128, T]), ("dyTr", [128, T]),
                            ("dv", [128, T * 4])):
            dbg[name] = nc.dram_tensor(name, shape, F16,
                                       kind="ExternalOutput").ap()

    with TileContext(nc) as tc:
        with (
            tc.tile_pool(name="wpool", bufs=1) as wpool,
            tc.tile_pool(name="xbpool", bufs=2) as xbpool,
            tc.tile_pool(name="qk", bufs=1) as qkpool,
            tc.tile_pool(name="pf", bufs=2) as pfpool,
            tc.tile_pool(name="pr", bufs=2) as prpool,
            tc.tile_pool(name="opool", bufs=2) as opool,
            tc.tile_pool(name="rpool", bufs=2) as rpool,
            tc.tile_pool(name="ps_m", bufs=2, space="PSUM") as ps_m,
            tc.tile_pool(name="ps_s", bufs=2, space="PSUM") as ps_s,
            tc.tile_pool(name="ps_y", bufs=1, space="PSUM") as ps_y,
        ):
            pools = (wpool, xbpool, qkpool, pfpool, prpool, opool, rpool,
                     ps_m, ps_s, ps_y)
            for _ in range(reps):
                _emit_body(nc, pools, aps, dbg=dbg)

    nc.compile()
    return nc


_NC_CACHE = {}


def _get_nc(reps=1):
    if reps not in _NC_CACHE:
        _NC_CACHE[reps] = _build_nc(reps)
    return _NC_CACHE[reps]


_MASKS = None


def make_in_maps(x, w_qkv_full, w_qk_red, w_v_red, w_proj):
    global _MASKS
    if _MASKS is None:
        _MASKS = np.ascontiguousarray(host_masks().reshape(128, 256))
    x = np.asarray(x, np.float32)
    w_qkv_full = np.asarray(w_qkv_full, np.float32)
    w_qk_red = np.asarray(w_qk_red, np.float32)
    w_v_red = np.asarray(w_v_red, np.float32)
    w_proj = np.asarray(w_proj, np.float32)
    sf = np.float32(1.0 / np.sqrt(HDIM))
    sr = np.float32(1.0 / np.sqrt(RDIM))
    in_maps = []
    for c in range(N_CORES):
        b, g = divmod(c, 4)
        hA, hB = 2 * g, 2 * g + 1
        wq = np.concatenate([w_qkv_full[:, 64 * hA:64 * hA + 64],
                             w_qkv_full[:, 64 * hB:64 * hB + 64]], 1) * sf
        wk = np.concatenate([w_qkv_full[:, 512 + 64 * hA:512 + 64 * hA + 64],
                             w_qkv_full[:, 512 + 64 * hB:512 + 64 * hB + 64]], 1)
        # packed reduced projection: rows 0:32 krA | 32:64 krB | qrA | qrB
        wqkr = np.concatenate(
            [w_qk_red[:, 256 + 32 * hA:256 + 32 * hA + 32],
             w_qk_red[:, 256 + 32 * hB:256 + 32 * hB + 32],
             w_qk_red[:, 32 * hA:32 * hA + 32] * sr,
             w_qk_red[:, 32 * hB:32 * hB + 32] * sr], 1)
        wv = np.concatenate([w_qkv_full[:, 1024 + 64 * hA:1024 + 64 * hA + 64],
                             w_qkv_full[:, 1024 + 64 * hB:1024 + 64 * hB + 64],
                             w_v_red[:, 64 * hA:64 * hA + 64],
                             w_v_red[:, 64 * hB:64 * hB + 64]], 1)
        wp = np.concatenate([w_proj[64 * hA:64 * hA + 64, :],
                             w_proj[64 * hB:64 * hB + 64, :],
                             w_proj[512 + 64 * hA:512 + 64 * hA + 64, :],
                             w_proj[512 + 64 * hB:512 + 64 * hB + 64, :]], 0)
        wqkv = np.concatenate([wq, wk, wqkr, wv], 1)
        in_maps.append({
            "xT": np.ascontiguousarray(x[b].T).astype(np.float16),
            "wqkv": np.ascontiguousarray(wqkv).astype(np.float16),
            "wproj": np.ascontiguousarray(wp).astype(np.float16),
            "masks": _MASKS,
        })
    return in_maps


def kernel(x, w_qkv_full, w_qk_red, w_v_red, w_proj):
    nc = _get_nc()
    in_maps = make_in_maps(x, w_qkv_full, w_qk_red, w_v_red, w_proj)
    r = bass_utils.run_bass_kernel_spmd(nc, in_maps,
                                        core_ids=list(range(N_CORES)),
                                        trace=False)
    outs = [r.results[c]["o"] for c in range(N_CORES)]
    y = np.zeros((B, T, C), np.float32)
    for b in range(B):
        for j in range(4):
            y[b] += np.asarray(outs[4 * b + j], np.float32)
    return y
